# revision 1
# baseline (speedup 1.0000x reference)
"""Multi-head causal self-attention (B=32, S=512, E=768, H=12, D=64) on 8 TRN2 cores.

Sharding: pure data-parallel over batch (4 batches per core), no collectives.

Per-core layout strategy:
  - x is fed pre-transposed (feature-major) as xT [E, 2048tok].
  - Q^T, K^T are computed feature-major per head-pair (feature tile == head
    pair):  QT_hp = Wq[:, hp].T @ xT   (lhsT=Wq slice, rhs=xT)
  - V is computed token-major with an extra all-ones column per head
    ("V_aug" [tok, H*(D+1)]); the ones column makes the P@V matmul also
    produce the softmax denominators.
  - scores^T[k,q] = K Q^T computed per (head, k-tile of 128 tokens) with the
    causal-trimmed q range [128*i, 512), both heads of a pair packed into the
    128x128 PE array via tile_position row groups.
  - exp() on ScalarE reads score PSUM directly (1/sqrt(D) folded into exp's
    scale), both heads in one call; the causal mask is a post-exp 0/1
    multiply of just the diagonal 128x128 block on VectorE, kept OFF the
    PE->ACT critical path.
  - P@V: out[q, D+1] accumulated over k-tiles i<=j in PSUM; reciprocal of
    column D (the ones-column sum = softmax denominator) normalizes via a
    ScalarE copy with per-partition scale.
  - Y (token-major) is transposed 128x128 via TensorE back to feature-major
    for the output projection, which lands token-major for a contiguous DMA.
  - Emission is software-pipelined (scores of head-pair hp+1 before the PV
    block of hp; next batch's xT DMA prefetched mid-batch) so the in-order
    engine streams always have independent matmuls to hide the cross-engine
    softmax chains.
  - Matmul operands use float32r (single-pass relaxed fp32, 4x PE throughput,
    ~2e-4 absmax-relative error end to end). Set BASS_MM_F32=1 for strict
    fp32 (~2.5x slower, ~2e-6 error).
"""

import os
import sys

import numpy as np

for _p in ("/opt/trn_rl_repo", "/opt/trn_rl_repo/concourse"):
    if _p not in sys.path:
        sys.path.insert(0, _p)

import concourse.bass as bass
import concourse.bacc as bacc
import concourse.mybir as mybir
import concourse.tile as tile

P = 128
E = 768
S = 512
H = 12
D = 64
HP = H // 2          # head pairs
KT = E // P          # 6 feature k-tiles
N_CORES = 8
B_FULL = 32
B_CORE = B_FULL // N_CORES   # 4 batches per core
TOK = B_CORE * S             # 2048 tokens per core
ST = S // P                  # 4 token tiles per sequence
NEG = -1.0e6                 # pre-scale mask bias; exp(0.125 * -1e6) == 0
F32 = mybir.dt.float32

# number of 384-wide chunks for the V / O projections
CH = 2
CHW = E // CH  # 384


def build_program(with_bias: bool, repeat: int = 1, hw_loop: bool = False,
                  r_proj: bool = False, r_scores: bool = False, phases: int = 3):
    PDT = mybir.dt.float32r if r_proj else F32   # proj operands (x, weights, y)
    SDT = mybir.dt.float32r if r_scores else F32  # scores operands (qt, kt)
    nc = bacc.Bacc(None)

    xt_d = nc.dram_tensor("xt", [E, TOK], PDT, kind="ExternalInput")
    w_d = {
        n: nc.dram_tensor(n, [E, E], PDT, kind="ExternalInput")
        for n in ("wq", "wk", "wv", "wo")
    }
    consts_d = nc.dram_tensor("consts", [P, 3 * P], F32, kind="ExternalInput")
    if with_bias:
        bqk_d = nc.dram_tensor("bqk", [P, 2 * KT], F32, kind="ExternalInput")
        bv_d = nc.dram_tensor("bvb", [P, H * (D + 1)], F32, kind="ExternalInput")
        bo_d = nc.dram_tensor("bob", [P, E], F32, kind="ExternalInput")
    y_d = nc.dram_tensor("y", [TOK, E], F32, kind="ExternalOutput")

    with tile.TileContext(nc) as tc:
        with (
            tc.tile_pool(name="wpool", bufs=1) as wpool,
            tc.tile_pool(name="xpool", bufs=2) as xpool,
            tc.tile_pool(name="qkpool", bufs=int(os.environ.get("B_QK", "3"))) as qkpool,
            tc.tile_pool(name="vpool", bufs=int(os.environ.get("B_VS", "2"))) as vpool,
            tc.tile_pool(name="ppool", bufs=int(os.environ.get("B_PT", "8"))) as ppool,
            tc.tile_pool(name="mdpool", bufs=int(os.environ.get("B_MD", "8"))) as mdpool,
            tc.tile_pool(name="ypool", bufs=4) as ypool,
            tc.tile_pool(name="ytpool", bufs=2) as ytpool,
            tc.tile_pool(name="opool", bufs=2) as opool,
            tc.tile_pool(name="rpool", bufs=4) as rpool,
            tc.tile_pool(name="ps_mm", bufs=int(os.environ.get("B_MM", "3")), space="PSUM") as ps_mm,
            tc.tile_pool(name="ps_sc", bufs=int(os.environ.get("B_SC", "1")), space="PSUM") as ps_sc,
            tc.tile_pool(name="ps_pv", bufs=int(os.environ.get("B_PV", "2")), space="PSUM") as ps_pv,
            tc.tile_pool(name="ps_yt", bufs=int(os.environ.get("B_YT", "1")), space="PSUM") as ps_yt,
        ):
            # ---- persistent constants ----
            w_sb = {}
            for n in ("wq", "wk", "wv", "wo"):
                t = wpool.tile([P, KT, E], PDT, tag=n)
                nc.sync.dma_start(t[:], w_d[n][:].rearrange("(ko ki) m -> ki ko m", ki=P))
                w_sb[n] = t
            cons = wpool.tile([P, 3 * P], F32, tag="consts")  # masks stay f32
            nc.sync.dma_start(cons[:], consts_d[:])
            ident = cons[:, 0:P]
            mask01 = cons[:, 2 * P : 3 * P]
            if with_bias:
                bqk = wpool.tile([P, 2 * KT], F32, tag="bqk")
                nc.sync.dma_start(bqk[:], bqk_d[:])
                bvb = wpool.tile([P, H * (D + 1)], F32, tag="bvb")
                nc.sync.dma_start(bvb[:], bv_d[:])
                bob = wpool.tile([P, E], F32, tag="bob")
                nc.sync.dma_start(bob[:], bo_d[:])

            xt_r = xt_d[:].rearrange("(ko ki) t -> ki ko t", ki=P)

            xts_t = {}

            def load(pos, b):
                tok0 = (b % B_CORE) * S
                xts = xpool.tile([P, KT, S], PDT, tag="xts")
                nc.sync.dma_start(xts[:], xt_r[:, :, tok0 : tok0 + S])
                xts_t[pos] = xts

            def vproj(b, xts):
                # ---- V projection (token-major, augmented with ones cols) ----
                vs = []
                for tt in range(ST):
                    v_t = vpool.tile([P, H, D + 1], F32, tag=f"vs{tt}")
                    nc.gpsimd.memset(v_t[:, :, D : D + 1], 1.0)
                    for ch in range(CH):
                        ps = ps_mm.tile([P, S], F32, tag="mm")
                        psc = ps[:, :CHW]
                        for k in range(KT):
                            nc.tensor.matmul(
                                psc,
                                xts[:, k, tt * P : (tt + 1) * P],
                                w_sb["wv"][:, k, ch * CHW : (ch + 1) * CHW],
                                start=(k == 0),
                                stop=(k == KT - 1),
                            )
                        hpc = CHW // D  # heads per chunk (6)
                        dst = v_t[:, ch * hpc : (ch + 1) * hpc, 0:D]
                        nc.any.tensor_copy(out=dst, in_=psc.rearrange("p (h d) -> p h d", d=D))
                    if with_bias:
                        nc.vector.tensor_add(
                            out=v_t[:],
                            in0=v_t[:],
                            in1=bvb[:].rearrange("p (h d) -> p h d", d=D + 1),
                        )
                    vs.append(v_t)
                return vs

            def qk_scores(b, xts, hp):
                # Q^T / K^T for this head pair (feature tile hp)
                qk = {}
                for name, tag in (("wq", "qt"), ("wk", "kt")):
                    dst = qkpool.tile([P, S], SDT, tag=tag)
                    ps = ps_mm.tile([P, S], F32, tag="mm")
                    for k in range(KT):
                        nc.tensor.matmul(
                            ps[:],
                            w_sb[name][:, k, hp * P : (hp + 1) * P],
                            xts[:, k, :],
                            start=(k == 0),
                            stop=(k == KT - 1),
                        )
                    if with_bias:
                        col = (0 if name == "wq" else KT) + hp
                        nc.vector.tensor_scalar_add(
                            dst[:], ps[:], bqk[:, col : col + 1]
                        )
                    else:
                        nc.any.tensor_copy(out=dst[:], in_=ps[:])
                    qk[tag] = dst
                qt, kt = qk["qt"], qk["kt"]

                # scores^T + exp, causal-trimmed per k-tile
                pts = []  # pts[i] = exp(scores^T) [P, 2, Nq] (heads of pair)
                for i in range(ST):
                    nq = S - i * P
                    qoff = i * P
                    ps = ps_sc.tile([P, 2, S], F32, tag="sc")
                    for hh in range(2):
                        ro = hh * D
                        nc.tensor.matmul(
                            ps[:, hh, 0:nq],
                            kt[ro : ro + D, i * P : (i + 1) * P],
                            qt[ro : ro + D, qoff:S],
                            start=True,
                            stop=True,
                            tile_position=(ro, 0),
                        )
                    pt = ppool.tile([P, 2, S], F32, tag="pt")
                    nc.scalar.activation(
                        pt[:, :, 0:nq],
                        ps[:, :, 0:nq],
                        mybir.ActivationFunctionType.Exp,
                        scale=0.125,
                    )
                    # causal mask: zero the upper triangle of the diagonal
                    # block, off the PE->ACT critical path (VectorE, post-exp)
                    md = mdpool.tile([P, 2, P], F32, tag="md")
                    nc.vector.tensor_mul(
                        out=md[:], in0=pt[:, :, 0:P],
                        in1=mask01[:, None, :].to_broadcast((P, 2, P)),
                    )
                    pts.append((pt, md))
                return pts

            def pv_block(hp, pts, vs, yt):
                # P @ V_aug accumulated over k-tiles, then normalize,
                # then transpose Y back to feature-major.
                for j in range(ST):
                    yst = ypool.tile([P, P], F32, tag="yst")
                    for hh in range(2):
                        h = 2 * hp + hh
                        pv = ps_pv.tile([P, D + 1], F32, tag="pv")
                        for i in range(j + 1):
                            pt, md = pts[i]
                            lhsT = (
                                md[:, hh, :]
                                if i == j
                                else pt[:, hh, (j - i) * P : (j - i + 1) * P]
                            )
                            nc.tensor.matmul(
                                pv[:],
                                lhsT,
                                vs[i][:, h, :],
                                start=(i == 0),
                                stop=(i == j),
                            )
                        r = rpool.tile([P, 1], F32, tag="r")
                        nc.vector.reciprocal(r[:], pv[:, D : D + 1])
                        if os.environ.get("NORM_ACT", "1") == "1":
                            nc.scalar.activation(
                                yst[:, hh * D : (hh + 1) * D], pv[:, 0:D],
                                mybir.ActivationFunctionType.Copy, scale=r[:],
                            )
                        else:
                            nc.vector.tensor_scalar_mul(
                                yst[:, hh * D : (hh + 1) * D], pv[:, 0:D], r[:]
                            )
                    yt_ps = ps_yt.tile([P, P], F32, tag="ytp")
                    nc.tensor.transpose(yt_ps[:], yst[:], ident)
                    nc.any.tensor_copy(
                        out=yt[:, hp, j * P : (j + 1) * P], in_=yt_ps[:]
                    )

            def oproj_tt(b, yt, tt):
                tok0 = (b % B_CORE) * S
                if True:
                    o_sb = opool.tile([P, E], F32, tag="osb")
                    for ch in range(CH):
                        ps = ps_mm.tile([P, S], F32, tag="mm")
                        psc = ps[:, :CHW]
                        for k in range(KT):
                            nc.tensor.matmul(
                                psc,
                                yt[:, k, tt * P : (tt + 1) * P],
                                w_sb["wo"][:, k, ch * CHW : (ch + 1) * CHW],
                                start=(k == 0),
                                stop=(k == KT - 1),
                            )
                        nc.any.tensor_copy(
                            out=o_sb[:, ch * CHW : (ch + 1) * CHW], in_=psc
                        )
                    if with_bias:
                        nc.vector.tensor_add(out=o_sb[:], in0=o_sb[:], in1=bob[:])
                    nc.sync.dma_start(
                        y_d[tok0 + tt * P : tok0 + (tt + 1) * P, :], o_sb[:]
                    )

            def run_batches(batches):
                # Software-pipelined emission: scores of head-pair hp+1 are
                # emitted before the PV block of hp, so the tensor engine's
                # in-order stream always has matmuls to run while the
                # mask(DVE) -> exp(ACT) -> normalize(DVE) chains drain.
                load(0, batches[0])
                pending_o = None  # (b, yt) of the previous batch
                for idx, b in enumerate(batches):
                    xts = xts_t.pop(idx)
                    vs = vproj(b, xts)
                    yt = ytpool.tile([P, KT, S], PDT, tag="yt")
                    pts_next = qk_scores(b, xts, 0)
                    for hp in range(HP):
                        pts_cur = pts_next
                        # previous batch's output projection, one token tile
                        # at a time, spread through the PV chain gaps
                        if pending_o is not None and hp < ST:
                            oproj_tt(*pending_o, hp)
                        if hp == 2 and idx + 1 < len(batches):
                            load(idx + 1, batches[idx + 1])
                        if hp + 1 < HP:
                            pts_next = qk_scores(b, xts, hp + 1)
                        pv_block(hp, pts_cur, vs, yt)
                    pending_o = (b, yt)
                for tt in range(ST):
                    oproj_tt(*pending_o, tt)

            if hw_loop and repeat > 1:
                with tc.For_i(0, repeat, 1):
                    run_batches(list(range(B_CORE)))
            else:
                run_batches([b % B_CORE for b in range(B_CORE * repeat)])

    nc.compile()
    return nc


def _host_consts():
    ident = np.eye(P, dtype=np.float32)
    k_idx = np.arange(P, dtype=np.int64)[:, None]
    q_idx = np.arange(P, dtype=np.int64)[None, :]
    maskb = np.where(k_idx <= q_idx, 0.0, NEG).astype(np.float32)
    mask01 = (k_idx <= q_idx).astype(np.float32)
    return np.concatenate([ident, maskb, mask01], axis=1)  # [P, 3P]


_PROG_CACHE = {}


# fp32r (relaxed single-pass fp32 matmul, ~2e-4 rel err, 4x PE throughput) is
# used by default; set BASS_MM_F32=1 for strict fp32 matmuls (~2x slower).
USE_F32R = os.environ.get("BASS_MM_F32", "0") != "1"


def _get_program(with_bias: bool):
    if with_bias not in _PROG_CACHE:
        _PROG_CACHE[with_bias] = build_program(
            with_bias, r_proj=USE_F32R, r_scores=USE_F32R
        )
    return _PROG_CACHE[with_bias]


def make_in_maps(x, Wq, bq, Wk, bk, Wv, bv, Wo, bo, with_bias):
    consts = _host_consts()
    maps = []
    for c in range(N_CORES):
        xc = np.ascontiguousarray(
            x[c * B_CORE : (c + 1) * B_CORE]  # [B_CORE, S, E]
            .reshape(TOK, E)
            .T  # [E, TOK]
        ).astype(np.float32)
        m = {
            "xt": xc,
            "wq": np.ascontiguousarray(Wq, dtype=np.float32),
            "wk": np.ascontiguousarray(Wk, dtype=np.float32),
            "wv": np.ascontiguousarray(Wv, dtype=np.float32),
            "wo": np.ascontiguousarray(Wo, dtype=np.float32),
            "consts": consts,
        }
        if with_bias:
            bqk = np.concatenate(
                [np.asarray(bq).reshape(KT, P).T, np.asarray(bk).reshape(KT, P).T],
                axis=1,
            ).astype(np.float32)
            bvb = np.zeros((P, H, D + 1), np.float32)
            bvb[:, :, :D] = np.broadcast_to(np.asarray(bv).reshape(H, D), (P, H, D))
            m["bqk"] = np.ascontiguousarray(bqk)
            m["bvb"] = np.ascontiguousarray(bvb.reshape(P, H * (D + 1)))
            m["bob"] = np.ascontiguousarray(
                np.broadcast_to(np.asarray(bo, dtype=np.float32), (P, E))
            )
        maps.append(m)
    return maps


def kernel(x, Wq, bq, Wk, bk, Wv, bv, Wo, bo):
    from concourse.bass_utils import run_bass_kernel_spmd

    x = np.asarray(x, dtype=np.float32)
    with_bias = any(
        float(np.abs(np.asarray(b)).max()) != 0.0 for b in (bq, bk, bv, bo)
    )
    nc = _get_program(with_bias)
    in_maps = make_in_maps(x, Wq, bq, Wk, bk, Wv, bv, Wo, bo, with_bias)
    res = run_bass_kernel_spmd(nc, in_maps, core_ids=list(range(N_CORES)))
    out = np.empty((B_FULL, S, E), dtype=np.float32)
    for c in range(N_CORES):
        out[c * B_CORE : (c + 1) * B_CORE] = res.results[c]["y"].reshape(B_CORE, S, E)
    return out



# revision 7
# speedup vs baseline: 2.7757x; 2.7757x over previous
"""Multi-head causal self-attention (B=32, S=512, E=768, H=12, D=64) on 8 TRN2 cores.

Sharding: pure data-parallel over batch (4 batches per core), no collectives.

Per-core layout strategy:
  - x is fed pre-transposed (feature-major) as xT [E, 2048tok].
  - Q^T, K^T are computed feature-major per head-pair (feature tile == head
    pair):  QT_hp = Wq[:, hp].T @ xT   (lhsT=Wq slice, rhs=xT)
  - V is computed token-major with an extra all-ones column per head
    ("V_aug" [tok, H*(D+1)]); the ones column makes the P@V matmul also
    produce the softmax denominators.
  - scores^T[k,q] = K Q^T computed per (head, k-tile of 128 tokens) with the
    causal-trimmed q range [128*i, 512), both heads of a pair packed into the
    128x128 PE array via tile_position row groups.
  - exp() on ScalarE reads score PSUM directly (1/sqrt(D) folded into exp's
    scale), both heads in one call; the causal mask is a post-exp 0/1
    multiply of just the diagonal 128x128 block on VectorE, kept OFF the
    PE->ACT critical path.
  - P@V: out[q, D+1] accumulated over k-tiles i<=j in PSUM; reciprocal of
    column D (the ones-column sum = softmax denominator) normalizes via a
    ScalarE copy with per-partition scale.
  - Y (token-major) is transposed 128x128 via TensorE back to feature-major
    for the output projection, which lands token-major for a contiguous DMA.
  - Emission is software-pipelined (scores of head-pair hp+1 before the PV
    block of hp; next batch's xT DMA prefetched mid-batch) so the in-order
    engine streams always have independent matmuls to hide the cross-engine
    softmax chains.
  - Matmul operands use float32r (single-pass relaxed fp32, 4x PE throughput,
    ~2e-4 absmax-relative error end to end). Set BASS_MM_F32=1 for strict
    fp32 (~2.5x slower, ~2e-6 error).
"""

import os
import sys

import numpy as np

for _p in ("/opt/trn_rl_repo", "/opt/trn_rl_repo/concourse"):
    if _p not in sys.path:
        sys.path.insert(0, _p)

import concourse.bass as bass
import concourse.bacc as bacc
import concourse.mybir as mybir
import concourse.tile as tile

P = 128
E = 768
S = 512
H = 12
D = 64
HP = H // 2          # head pairs
KT = E // P          # 6 feature k-tiles
N_CORES = 8
B_FULL = 32
B_CORE = B_FULL // N_CORES   # 4 batches per core
TOK = B_CORE * S             # 2048 tokens per core
ST = S // P                  # 4 token tiles per sequence
NEG = -1.0e6                 # pre-scale mask bias; exp(0.125 * -1e6) == 0
F32 = mybir.dt.float32

# number of 384-wide chunks for the V / O projections
CH = 2
CHW = E // CH  # 384


def build_program(with_bias: bool, repeat: int = 1, hw_loop: bool = False,
                  r_proj: bool = False, r_scores: bool = False, phases: int = 3,
                  bf16: bool = False):
    if bf16:
        PDT = SDT = mybir.dt.bfloat16
    else:
        PDT = mybir.dt.float32r if r_proj else F32   # proj operands (x, weights, y)
        SDT = mybir.dt.float32r if r_scores else F32  # scores operands (qt, kt)
    ADT = mybir.dt.bfloat16 if bf16 else F32  # attention-side tiles (p, v, y)
    nc = bacc.Bacc(None)

    xt_d = nc.dram_tensor("xt", [E, TOK], PDT, kind="ExternalInput")
    w_d = {
        n: nc.dram_tensor(n, [E, E], PDT, kind="ExternalInput")
        for n in ("wq", "wk", "wv", "wo")
    }
    consts_d = nc.dram_tensor("consts", [P, 3 * P], F32, kind="ExternalInput")
    if with_bias:
        bqk_d = nc.dram_tensor("bqk", [P, 2 * KT], F32, kind="ExternalInput")
        bv_d = nc.dram_tensor("bvb", [P, H * (D + 1)], F32, kind="ExternalInput")
        bo_d = nc.dram_tensor("bob", [P, E], F32, kind="ExternalInput")
    y_d = nc.dram_tensor("y", [TOK, E], F32, kind="ExternalOutput")

    with tile.TileContext(nc) as tc:
        with (
            tc.tile_pool(name="wpool", bufs=1) as wpool,
            tc.tile_pool(name="xpool", bufs=2) as xpool,
            tc.tile_pool(name="qkpool", bufs=int(os.environ.get("B_QK", "3"))) as qkpool,
            tc.tile_pool(name="vpool", bufs=int(os.environ.get("B_VS", "2"))) as vpool,
            tc.tile_pool(name="ppool", bufs=int(os.environ.get("B_PT", "8"))) as ppool,
            tc.tile_pool(name="mdpool", bufs=int(os.environ.get("B_MD", "8"))) as mdpool,
            tc.tile_pool(name="ypool", bufs=4) as ypool,
            tc.tile_pool(name="ytpool", bufs=2) as ytpool,
            tc.tile_pool(name="opool", bufs=2) as opool,
            tc.tile_pool(name="rpool", bufs=4) as rpool,
            tc.tile_pool(name="ps_mm", bufs=int(os.environ.get("B_MM", "3")), space="PSUM") as ps_mm,
            tc.tile_pool(name="ps_sc", bufs=int(os.environ.get("B_SC", "1")), space="PSUM") as ps_sc,
            tc.tile_pool(name="ps_pv", bufs=int(os.environ.get("B_PV", "2")), space="PSUM") as ps_pv,
            tc.tile_pool(name="ps_yt", bufs=int(os.environ.get("B_YT", "1")), space="PSUM") as ps_yt,
        ):
            # ---- persistent constants ----
            w_sb = {}
            for n in ("wq", "wk", "wv", "wo"):
                t = wpool.tile([P, KT, E], PDT, tag=n)
                nc.sync.dma_start(t[:], w_d[n][:].rearrange("(ko ki) m -> ki ko m", ki=P))
                w_sb[n] = t
            cons = wpool.tile([P, 3 * P], F32, tag="consts")  # masks stay f32
            nc.sync.dma_start(cons[:], consts_d[:])
            ident = cons[:, 0:P]
            mask01 = cons[:, 2 * P : 3 * P]
            if bf16:
                # bf16 copies so PE transpose / DVE mask-mul operands match
                consb = wpool.tile([P, 2 * P], mybir.dt.bfloat16, tag="consb")
                nc.any.tensor_copy(out=consb[:, 0:P], in_=ident)
                nc.any.tensor_copy(out=consb[:, P : 2 * P], in_=mask01)
                ident = consb[:, 0:P]
                mask01 = consb[:, P : 2 * P]
            if with_bias:
                bqk = wpool.tile([P, 2 * KT], F32, tag="bqk")
                nc.sync.dma_start(bqk[:], bqk_d[:])
                bvb = wpool.tile([P, H * (D + 1)], F32, tag="bvb")
                nc.sync.dma_start(bvb[:], bv_d[:])
                bob = wpool.tile([P, E], F32, tag="bob")
                nc.sync.dma_start(bob[:], bo_d[:])

            xt_r = xt_d[:].rearrange("(ko ki) t -> ki ko t", ki=P)

            xts_t = {}

            def load(pos, b):
                tok0 = (b % B_CORE) * S
                xts = xpool.tile([P, KT, S], PDT, tag="xts")
                nc.sync.dma_start(xts[:], xt_r[:, :, tok0 : tok0 + S])
                xts_t[pos] = xts

            def vproj(b, xts):
                # ---- V projection (token-major, augmented with ones cols) ----
                vs = []
                for tt in range(ST):
                    v_t = vpool.tile([P, H, D + 1], ADT, tag=f"vs{tt}")
                    nc.gpsimd.memset(v_t[:, :, D : D + 1], 1.0)
                    for ch in range(CH):
                        ps = ps_mm.tile([P, S], F32, tag="mm")
                        psc = ps[:, :CHW]
                        for k in range(KT):
                            nc.tensor.matmul(
                                psc,
                                xts[:, k, tt * P : (tt + 1) * P],
                                w_sb["wv"][:, k, ch * CHW : (ch + 1) * CHW],
                                start=(k == 0),
                                stop=(k == KT - 1),
                            )
                        hpc = CHW // D  # heads per chunk (6)
                        dst = v_t[:, ch * hpc : (ch + 1) * hpc, 0:D]
                        nc.any.tensor_copy(out=dst, in_=psc.rearrange("p (h d) -> p h d", d=D))
                    if with_bias:
                        nc.vector.tensor_add(
                            out=v_t[:],
                            in0=v_t[:],
                            in1=bvb[:].rearrange("p (h d) -> p h d", d=D + 1),
                        )
                    vs.append(v_t)
                return vs

            def qk_scores(b, xts, hp):
                # Q^T / K^T for this head pair (feature tile hp)
                qk = {}
                for name, tag in (("wq", "qt"), ("wk", "kt")):
                    dst = qkpool.tile([P, S], SDT, tag=tag)
                    ps = ps_mm.tile([P, S], F32, tag="mm")
                    for k in range(KT):
                        nc.tensor.matmul(
                            ps[:],
                            w_sb[name][:, k, hp * P : (hp + 1) * P],
                            xts[:, k, :],
                            start=(k == 0),
                            stop=(k == KT - 1),
                        )
                    if with_bias:
                        col = (0 if name == "wq" else KT) + hp
                        nc.vector.tensor_scalar_add(
                            dst[:], ps[:], bqk[:, col : col + 1]
                        )
                    else:
                        nc.any.tensor_copy(out=dst[:], in_=ps[:])
                    qk[tag] = dst
                qt, kt = qk["qt"], qk["kt"]

                # scores^T + exp, causal-trimmed per k-tile
                pts = []  # pts[i] = exp(scores^T) [P, 2, Nq] (heads of pair)
                for i in range(ST):
                    nq = S - i * P
                    qoff = i * P
                    ps = ps_sc.tile([P, 2, S], F32, tag="sc")
                    for hh in range(2):
                        ro = hh * D
                        nc.tensor.matmul(
                            ps[:, hh, 0:nq],
                            kt[ro : ro + D, i * P : (i + 1) * P],
                            qt[ro : ro + D, qoff:S],
                            start=True,
                            stop=True,
                            tile_position=(ro, 0),
                        )
                    pt = ppool.tile([P, 2, S], ADT, tag="pt")
                    nc.scalar.activation(
                        pt[:, :, 0:nq],
                        ps[:, :, 0:nq],
                        mybir.ActivationFunctionType.Exp,
                        scale=0.125,
                    )
                    # causal mask: zero the upper triangle of the diagonal
                    # block, off the PE->ACT critical path (VectorE, post-exp)
                    md = mdpool.tile([P, 2, P], ADT, tag="md")
                    nc.vector.tensor_mul(
                        out=md[:], in0=pt[:, :, 0:P],
                        in1=mask01[:, None, :].to_broadcast((P, 2, P)),
                    )
                    pts.append((pt, md))
                return pts

            def pv_block(hp, pts, vs, yt):
                # P @ V_aug accumulated over k-tiles, then normalize,
                # then transpose Y back to feature-major.
                for j in range(ST):
                    yst = ypool.tile([P, P], ADT, tag="yst")
                    for hh in range(2):
                        h = 2 * hp + hh
                        pv = ps_pv.tile([P, D + 1], F32, tag="pv")
                        for i in range(j + 1):
                            pt, md = pts[i]
                            lhsT = (
                                md[:, hh, :]
                                if i == j
                                else pt[:, hh, (j - i) * P : (j - i + 1) * P]
                            )
                            nc.tensor.matmul(
                                pv[:],
                                lhsT,
                                vs[i][:, h, :],
                                start=(i == 0),
                                stop=(i == j),
                            )
                        r = rpool.tile([P, 1], F32, tag="r")
                        nc.vector.reciprocal(r[:], pv[:, D : D + 1])
                        if os.environ.get("NORM_ACT", "1") == "1":
                            nc.scalar.activation(
                                yst[:, hh * D : (hh + 1) * D], pv[:, 0:D],
                                mybir.ActivationFunctionType.Copy, scale=r[:],
                            )
                        else:
                            nc.vector.tensor_scalar_mul(
                                yst[:, hh * D : (hh + 1) * D], pv[:, 0:D], r[:]
                            )
                    yt_ps = ps_yt.tile([P, P], ADT, tag="ytp")
                    nc.tensor.transpose(yt_ps[:], yst[:], ident)
                    nc.any.tensor_copy(
                        out=yt[:, hp, j * P : (j + 1) * P], in_=yt_ps[:]
                    )

            def oproj_tt(b, yt, tt):
                tok0 = (b % B_CORE) * S
                if True:
                    o_sb = opool.tile([P, E], F32, tag="osb")
                    for ch in range(CH):
                        ps = ps_mm.tile([P, S], F32, tag="mm")
                        psc = ps[:, :CHW]
                        for k in range(KT):
                            nc.tensor.matmul(
                                psc,
                                yt[:, k, tt * P : (tt + 1) * P],
                                w_sb["wo"][:, k, ch * CHW : (ch + 1) * CHW],
                                start=(k == 0),
                                stop=(k == KT - 1),
                            )
                        nc.any.tensor_copy(
                            out=o_sb[:, ch * CHW : (ch + 1) * CHW], in_=psc
                        )
                    if with_bias:
                        nc.vector.tensor_add(out=o_sb[:], in0=o_sb[:], in1=bob[:])
                    nc.sync.dma_start(
                        y_d[tok0 + tt * P : tok0 + (tt + 1) * P, :], o_sb[:]
                    )

            def run_batches(batches):
                # Software-pipelined emission: scores of head-pair hp+1 are
                # emitted before the PV block of hp, so the tensor engine's
                # in-order stream always has matmuls to run while the
                # mask(DVE) -> exp(ACT) -> normalize(DVE) chains drain.
                load(0, batches[0])
                pending_o = None  # (b, yt) of the previous batch
                for idx, b in enumerate(batches):
                    xts = xts_t.pop(idx)
                    vs = vproj(b, xts)
                    yt = ytpool.tile([P, KT, S], PDT, tag="yt")
                    pts_next = qk_scores(b, xts, 0)
                    for hp in range(HP):
                        pts_cur = pts_next
                        # previous batch's output projection, one token tile
                        # at a time, spread through the PV chain gaps
                        if pending_o is not None and hp < ST:
                            oproj_tt(*pending_o, hp)
                        if hp == 2 and idx + 1 < len(batches):
                            load(idx + 1, batches[idx + 1])
                        if hp + 1 < HP:
                            pts_next = qk_scores(b, xts, hp + 1)
                        pv_block(hp, pts_cur, vs, yt)
                    pending_o = (b, yt)
                for tt in range(ST):
                    oproj_tt(*pending_o, tt)

            if hw_loop and repeat > 1:
                with tc.For_i(0, repeat, 1):
                    run_batches(list(range(B_CORE)))
            else:
                run_batches([b % B_CORE for b in range(B_CORE * repeat)])

    nc.compile()
    return nc


def _host_consts():
    ident = np.eye(P, dtype=np.float32)
    k_idx = np.arange(P, dtype=np.int64)[:, None]
    q_idx = np.arange(P, dtype=np.int64)[None, :]
    maskb = np.where(k_idx <= q_idx, 0.0, NEG).astype(np.float32)
    mask01 = (k_idx <= q_idx).astype(np.float32)
    return np.concatenate([ident, maskb, mask01], axis=1)  # [P, 3P]


_PROG_CACHE = {}


# Matmul operand dtype: bf16 (default; 1 cycle/row on PE for ALL free-dim
# sizes, half the DMA bytes, ~2e-3 end-to-end rel err) or fp32r via
# BASS_MM_F32R=1 (~2e-4 err but 4 cycles/row whenever the matmul free dim
# < 256 — the PV and diagonal-score matmuls) or strict f32 via BASS_MM_F32=1.
USE_BF16 = (
    os.environ.get("BASS_MM_F32", "0") != "1"
    and os.environ.get("BASS_MM_F32R", "0") != "1"
)
USE_F32R = not USE_BF16 and os.environ.get("BASS_MM_F32", "0") != "1"


def _get_program(with_bias: bool):
    if with_bias not in _PROG_CACHE:
        _PROG_CACHE[with_bias] = build_program(
            with_bias, r_proj=USE_F32R, r_scores=USE_F32R, bf16=USE_BF16
        )
    return _PROG_CACHE[with_bias]


def make_in_maps(x, Wq, bq, Wk, bk, Wv, bv, Wo, bo, with_bias):
    if USE_BF16:
        import ml_dtypes

        in_dt = ml_dtypes.bfloat16
    else:
        in_dt = np.float32
    consts = _host_consts()
    maps = []
    for c in range(N_CORES):
        xc = np.ascontiguousarray(
            x[c * B_CORE : (c + 1) * B_CORE]  # [B_CORE, S, E]
            .reshape(TOK, E)
            .T  # [E, TOK]
        ).astype(in_dt)
        m = {
            "xt": xc,
            "wq": np.ascontiguousarray(np.asarray(Wq, dtype=np.float32).astype(in_dt)),
            "wk": np.ascontiguousarray(np.asarray(Wk, dtype=np.float32).astype(in_dt)),
            "wv": np.ascontiguousarray(np.asarray(Wv, dtype=np.float32).astype(in_dt)),
            "wo": np.ascontiguousarray(np.asarray(Wo, dtype=np.float32).astype(in_dt)),
            "consts": consts,
        }
        if with_bias:
            bqk = np.concatenate(
                [np.asarray(bq).reshape(KT, P).T, np.asarray(bk).reshape(KT, P).T],
                axis=1,
            ).astype(np.float32)
            bvb = np.zeros((P, H, D + 1), np.float32)
            bvb[:, :, :D] = np.broadcast_to(np.asarray(bv).reshape(H, D), (P, H, D))
            m["bqk"] = np.ascontiguousarray(bqk)
            m["bvb"] = np.ascontiguousarray(bvb.reshape(P, H * (D + 1)))
            m["bob"] = np.ascontiguousarray(
                np.broadcast_to(np.asarray(bo, dtype=np.float32), (P, E))
            )
        maps.append(m)
    return maps


def kernel(x, Wq, bq, Wk, bk, Wv, bv, Wo, bo):
    from concourse.bass_utils import run_bass_kernel_spmd

    x = np.asarray(x, dtype=np.float32)
    with_bias = any(
        float(np.abs(np.asarray(b)).max()) != 0.0 for b in (bq, bk, bv, bo)
    )
    nc = _get_program(with_bias)
    in_maps = make_in_maps(x, Wq, bq, Wk, bk, Wv, bv, Wo, bo, with_bias)
    res = run_bass_kernel_spmd(nc, in_maps, core_ids=list(range(N_CORES)))
    out = np.empty((B_FULL, S, E), dtype=np.float32)
    for c in range(N_CORES):
        out[c * B_CORE : (c + 1) * B_CORE] = res.results[c]["y"].reshape(B_CORE, S, E)
    return out



# revision 12
# speedup vs baseline: 3.1819x; 1.1464x over previous
"""Multi-head causal self-attention (B=32, S=512, E=768, H=12, D=64) on 8 TRN2 cores.

Sharding: pure data-parallel over batch (4 batches per core), no collectives.

Per-core layout strategy:
  - x is fed pre-transposed (feature-major) as xT [E, 2048tok].
  - Q^T, K^T are computed feature-major per head-pair (feature tile == head
    pair):  QT_hp = Wq[:, hp].T @ xT   (lhsT=Wq slice, rhs=xT)
  - V is computed token-major with an extra all-ones column per head
    ("V_aug" [tok, H*(D+1)]); the ones column makes the P@V matmul also
    produce the softmax denominators.
  - scores^T[k,q] = K Q^T computed per (head, k-tile of 128 tokens) with the
    causal-trimmed q range [128*i, 512), both heads of a pair packed into the
    128x128 PE array via tile_position row groups.
  - exp() on ScalarE reads score PSUM directly (1/sqrt(D) folded into exp's
    scale), both heads in one call; the causal mask is a post-exp 0/1
    multiply of just the diagonal 128x128 block on VectorE, kept OFF the
    PE->ACT critical path.
  - P@V: out[q, D+1] accumulated over k-tiles i<=j in PSUM; reciprocal of
    column D (the ones-column sum = softmax denominator) normalizes via a
    ScalarE copy with per-partition scale.
  - Y (token-major) is transposed 128x128 via TensorE back to feature-major
    for the output projection, which lands token-major for a contiguous DMA.
  - Emission is software-pipelined (scores of head-pair hp+1 before the PV
    block of hp; next batch's xT DMA prefetched mid-batch) so the in-order
    engine streams always have independent matmuls to hide the cross-engine
    softmax chains.
  - Matmul operands use float32r (single-pass relaxed fp32, 4x PE throughput,
    ~2e-4 absmax-relative error end to end). Set BASS_MM_F32=1 for strict
    fp32 (~2.5x slower, ~2e-6 error).
"""

import os
import sys

import numpy as np

for _p in ("/opt/trn_rl_repo", "/opt/trn_rl_repo/concourse"):
    if _p not in sys.path:
        sys.path.insert(0, _p)

import concourse.bass as bass
import concourse.bacc as bacc
import concourse.mybir as mybir
import concourse.tile as tile

P = 128
E = 768
S = 512
H = 12
D = 64
HP = H // 2          # head pairs
KT = E // P          # 6 feature k-tiles
N_CORES = 8
B_FULL = 32
B_CORE = B_FULL // N_CORES   # 4 batches per core
TOK = B_CORE * S             # 2048 tokens per core
ST = S // P                  # 4 token tiles per sequence
NEG = -1.0e6                 # pre-scale mask bias; exp(0.125 * -1e6) == 0
F32 = mybir.dt.float32

# number of 384-wide chunks for the V / O projections
CH = 2
CHW = E // CH  # 384


def build_program(with_bias: bool, repeat: int = 1, hw_loop: bool = False,
                  r_proj: bool = False, r_scores: bool = False, phases: int = 3,
                  bf16: bool = False):
    if bf16:
        PDT = SDT = mybir.dt.bfloat16
    else:
        PDT = mybir.dt.float32r if r_proj else F32   # proj operands (x, weights, y)
        SDT = mybir.dt.float32r if r_scores else F32  # scores operands (qt, kt)
    ADT = mybir.dt.bfloat16 if bf16 else F32  # attention-side tiles (p, v, y)
    nc = bacc.Bacc(None)

    xt_d = nc.dram_tensor("xt", [E, TOK], PDT, kind="ExternalInput")
    w_d = {
        n: nc.dram_tensor(n, [E, E], PDT, kind="ExternalInput")
        for n in ("wq", "wk", "wv", "wo")
    }
    consts_d = nc.dram_tensor("consts", [P, 3 * P], F32, kind="ExternalInput")
    if with_bias:
        bqk_d = nc.dram_tensor("bqk", [P, 2 * KT], F32, kind="ExternalInput")
        bv_d = nc.dram_tensor("bvb", [P, H * (D + 1)], F32, kind="ExternalInput")
        bo_d = nc.dram_tensor("bob", [P, E], F32, kind="ExternalInput")
    y_d = nc.dram_tensor("y", [TOK, E], F32, kind="ExternalOutput")

    with tile.TileContext(nc) as tc:
        with (
            tc.tile_pool(name="wpool", bufs=1) as wpool,
            tc.tile_pool(name="xpool", bufs=2) as xpool,
            tc.tile_pool(name="qkpool", bufs=int(os.environ.get("B_QK", "3"))) as qkpool,
            tc.tile_pool(name="vpool", bufs=int(os.environ.get("B_VS", "2"))) as vpool,
            tc.tile_pool(name="ppool", bufs=int(os.environ.get("B_PT", "8"))) as ppool,
            tc.tile_pool(name="mdpool", bufs=int(os.environ.get("B_MD", "8"))) as mdpool,
            tc.tile_pool(name="ypool", bufs=4) as ypool,
            tc.tile_pool(name="ytpool", bufs=2) as ytpool,
            tc.tile_pool(name="opool", bufs=2) as opool,
            tc.tile_pool(name="rpool", bufs=4) as rpool,
            tc.tile_pool(name="ps_mm", bufs=int(os.environ.get("B_MM", "3")), space="PSUM") as ps_mm,
            tc.tile_pool(name="ps_sc", bufs=int(os.environ.get("B_SC", "1")), space="PSUM") as ps_sc,
            tc.tile_pool(name="ps_pv", bufs=int(os.environ.get("B_PV", "2")), space="PSUM") as ps_pv,
            tc.tile_pool(name="ps_yt", bufs=int(os.environ.get("B_YT", "1")), space="PSUM") as ps_yt,
        ):
            # ---- persistent constants ----
            w_sb = {}
            for n in ("wq", "wk", "wv", "wo"):
                t = wpool.tile([P, KT, E], PDT, tag=n)
                nc.sync.dma_start(t[:], w_d[n][:].rearrange("(ko ki) m -> ki ko m", ki=P))
                w_sb[n] = t
            cons = wpool.tile([P, 3 * P], F32, tag="consts")  # masks stay f32
            nc.sync.dma_start(cons[:], consts_d[:])
            ident = cons[:, 0:P]
            mask01 = cons[:, 2 * P : 3 * P]
            if bf16:
                # bf16 copies so PE transpose / DVE mask-mul operands match
                consb = wpool.tile([P, 2 * P], mybir.dt.bfloat16, tag="consb")
                nc.any.tensor_copy(out=consb[:, 0:P], in_=ident)
                nc.any.tensor_copy(out=consb[:, P : 2 * P], in_=mask01)
                ident = consb[:, 0:P]
                mask01 = consb[:, P : 2 * P]
            if with_bias:
                bqk = wpool.tile([P, 2 * KT], F32, tag="bqk")
                nc.sync.dma_start(bqk[:], bqk_d[:])
                bvb = wpool.tile([P, H * (D + 1)], F32, tag="bvb")
                nc.sync.dma_start(bvb[:], bv_d[:])
                bob = wpool.tile([P, E], F32, tag="bob")
                nc.sync.dma_start(bob[:], bo_d[:])

            xt_r = xt_d[:].rearrange("(ko ki) t -> ki ko t", ki=P)

            xts_t = {}

            def _cp(out, in_):
                # PSUM->SBUF copies: GPSIMD/Pool cannot read PSUM (walrus
                # verifier), so route to DVE (tensor_copy) or ACT (Copy).
                if os.environ.get("CPY_ENG", "dve") == "act":
                    nc.scalar.activation(
                        out, in_, mybir.ActivationFunctionType.Copy
                    )
                else:
                    nc.vector.tensor_copy(out=out, in_=in_)

            def load(pos, b):
                tok0 = (b % B_CORE) * S
                xts = xpool.tile([P, KT, S], PDT, tag="xts")
                nc.sync.dma_start(xts[:], xt_r[:, :, tok0 : tok0 + S])
                xts_t[pos] = xts

            def vproj(b, xts):
                # ---- V projection (token-major, augmented with ones cols) ----
                vs = []
                for tt in range(ST):
                    v_t = vpool.tile([P, H, D + 1], ADT, tag=f"vs{tt}")
                    nc.gpsimd.memset(v_t[:, :, D : D + 1], 1.0)
                    for ch in range(CH):
                        ps = ps_mm.tile([P, S], F32, tag="mm")
                        psc = ps[:, :CHW]
                        for k in range(KT):
                            nc.tensor.matmul(
                                psc,
                                xts[:, k, tt * P : (tt + 1) * P],
                                w_sb["wv"][:, k, ch * CHW : (ch + 1) * CHW],
                                start=(k == 0),
                                stop=(k == KT - 1),
                            )
                        hpc = CHW // D  # heads per chunk (6)
                        dst = v_t[:, ch * hpc : (ch + 1) * hpc, 0:D]
                        _cp(dst, psc.rearrange("p (h d) -> p h d", d=D))
                    if with_bias:
                        nc.vector.tensor_add(
                            out=v_t[:],
                            in0=v_t[:],
                            in1=bvb[:].rearrange("p (h d) -> p h d", d=D + 1),
                        )
                    vs.append(v_t)
                return vs

            def qk_scores(b, xts, hp):
                # Q^T / K^T for this head pair (feature tile hp)
                qk = {}
                for name, tag in (("wq", "qt"), ("wk", "kt")):
                    dst = qkpool.tile([P, S], SDT, tag=tag)
                    ps = ps_mm.tile([P, S], F32, tag="mm")
                    for k in range(KT):
                        nc.tensor.matmul(
                            ps[:],
                            w_sb[name][:, k, hp * P : (hp + 1) * P],
                            xts[:, k, :],
                            start=(k == 0),
                            stop=(k == KT - 1),
                        )
                    if with_bias:
                        col = (0 if name == "wq" else KT) + hp
                        nc.vector.tensor_scalar_add(
                            dst[:], ps[:], bqk[:, col : col + 1]
                        )
                    else:
                        _cp(dst[:], ps[:])
                    qk[tag] = dst
                qt, kt = qk["qt"], qk["kt"]

                # scores^T + exp, causal-trimmed per k-tile
                pts = []  # pts[i] = exp(scores^T) [P, 2, Nq] (heads of pair)
                sc_split = os.environ.get("SC_SPLIT", "0") == "1"
                for i in range(ST):
                    nq = S - i * P
                    qoff = i * P
                    pt = ppool.tile([P, 2, S], ADT, tag="pt")
                    if sc_split:
                        # per-head 1-bank score PSUM: finer matmul->exp
                        # pipelining and a smaller pool footprint
                        for hh in range(2):
                            ro = hh * D
                            ps1 = ps_sc.tile([P, S], F32, tag="sc")
                            nc.tensor.matmul(
                                ps1[:, 0:nq],
                                kt[ro : ro + D, i * P : (i + 1) * P],
                                qt[ro : ro + D, qoff:S],
                                start=True,
                                stop=True,
                                tile_position=(ro, 0),
                            )
                            nc.scalar.activation(
                                pt[:, hh, 0:nq],
                                ps1[:, 0:nq],
                                mybir.ActivationFunctionType.Exp,
                                scale=0.125,
                            )
                    else:
                        ps = ps_sc.tile([P, 2, S], F32, tag="sc")
                        for hh in range(2):
                            ro = hh * D
                            nc.tensor.matmul(
                                ps[:, hh, 0:nq],
                                kt[ro : ro + D, i * P : (i + 1) * P],
                                qt[ro : ro + D, qoff:S],
                                start=True,
                                stop=True,
                                tile_position=(ro, 0),
                            )
                        nc.scalar.activation(
                            pt[:, :, 0:nq],
                            ps[:, :, 0:nq],
                            mybir.ActivationFunctionType.Exp,
                            scale=0.125,
                        )
                    # causal mask: zero the upper triangle of the diagonal
                    # block, off the PE->ACT critical path (VectorE, post-exp)
                    md = mdpool.tile([P, 2, P], ADT, tag="md")
                    _mask_eng = (
                        nc.gpsimd
                        if os.environ.get("MASK_ENG", "pool") == "pool"
                        else nc.vector
                    )
                    _mask_eng.tensor_mul(
                        out=md[:], in0=pt[:, :, 0:P],
                        in1=mask01[:, None, :].to_broadcast((P, 2, P)),
                    )
                    pts.append((pt, md))
                return pts

            def pv_block(hp, pts, vs, yt):
                # P @ V_aug accumulated over k-tiles, then normalize,
                # then transpose Y back to feature-major.
                for j in range(ST):
                    yst = ypool.tile([P, P], ADT, tag="yst")
                    for hh in range(2):
                        h = 2 * hp + hh
                        pv = ps_pv.tile([P, D + 1], F32, tag="pv")
                        for i in range(j + 1):
                            pt, md = pts[i]
                            lhsT = (
                                md[:, hh, :]
                                if i == j
                                else pt[:, hh, (j - i) * P : (j - i + 1) * P]
                            )
                            nc.tensor.matmul(
                                pv[:],
                                lhsT,
                                vs[i][:, h, :],
                                start=(i == 0),
                                stop=(i == j),
                            )
                        r = rpool.tile([P, 1], F32, tag="r")
                        nc.vector.reciprocal(r[:], pv[:, D : D + 1])
                        # Normalize off ScalarE: keeping ACT exclusively on
                        # Exp avoids a 1283ns activation-table reload per
                        # Exp<->Copy switch (~63us/iter in the cost model).
                        norm_eng = os.environ.get("NORM_ENG", "dve")
                        if norm_eng == "act":
                            nc.scalar.activation(
                                yst[:, hh * D : (hh + 1) * D], pv[:, 0:D],
                                mybir.ActivationFunctionType.Copy, scale=r[:],
                            )
                        elif norm_eng == "dve":
                            nc.vector.tensor_scalar_mul(
                                yst[:, hh * D : (hh + 1) * D], pv[:, 0:D], r[:]
                            )
                        else:
                            nc.gpsimd.tensor_scalar_mul(
                                yst[:, hh * D : (hh + 1) * D], pv[:, 0:D], r[:]
                            )
                    yt_ps = ps_yt.tile([P, P], ADT, tag="ytp")
                    nc.tensor.transpose(yt_ps[:], yst[:], ident)
                    _cp(yt[:, hp, j * P : (j + 1) * P], yt_ps[:])

            def oproj_tt(b, yt, tt):
                tok0 = (b % B_CORE) * S
                if True:
                    o_sb = opool.tile([P, E], F32, tag="osb")
                    for ch in range(CH):
                        ps = ps_mm.tile([P, S], F32, tag="mm")
                        psc = ps[:, :CHW]
                        for k in range(KT):
                            nc.tensor.matmul(
                                psc,
                                yt[:, k, tt * P : (tt + 1) * P],
                                w_sb["wo"][:, k, ch * CHW : (ch + 1) * CHW],
                                start=(k == 0),
                                stop=(k == KT - 1),
                            )
                        _cp(o_sb[:, ch * CHW : (ch + 1) * CHW], psc)
                    if with_bias:
                        nc.vector.tensor_add(out=o_sb[:], in0=o_sb[:], in1=bob[:])
                    nc.sync.dma_start(
                        y_d[tok0 + tt * P : tok0 + (tt + 1) * P, :], o_sb[:]
                    )

            def run_batches(batches):
                # Software-pipelined emission: scores of head-pair hp+1 are
                # emitted before the PV block of hp, so the tensor engine's
                # in-order stream always has matmuls to run while the
                # mask(DVE) -> exp(ACT) -> normalize(DVE) chains drain.
                load(0, batches[0])
                pending_o = None  # (b, yt) of the previous batch
                for idx, b in enumerate(batches):
                    xts = xts_t.pop(idx)
                    vs = vproj(b, xts)
                    yt = ytpool.tile([P, KT, S], PDT, tag="yt")
                    pts_next = qk_scores(b, xts, 0)
                    for hp in range(HP):
                        pts_cur = pts_next
                        # previous batch's output projection, one token tile
                        # at a time, spread through the PV chain gaps
                        if pending_o is not None and hp < ST:
                            oproj_tt(*pending_o, hp)
                        if hp == 2 and idx + 1 < len(batches):
                            load(idx + 1, batches[idx + 1])
                        if hp + 1 < HP:
                            pts_next = qk_scores(b, xts, hp + 1)
                        pv_block(hp, pts_cur, vs, yt)
                    pending_o = (b, yt)
                for tt in range(ST):
                    oproj_tt(*pending_o, tt)

            if hw_loop and repeat > 1:
                with tc.For_i(0, repeat, 1):
                    run_batches(list(range(B_CORE)))
            else:
                run_batches([b % B_CORE for b in range(B_CORE * repeat)])

    nc.compile()
    return nc


def _host_consts():
    ident = np.eye(P, dtype=np.float32)
    k_idx = np.arange(P, dtype=np.int64)[:, None]
    q_idx = np.arange(P, dtype=np.int64)[None, :]
    maskb = np.where(k_idx <= q_idx, 0.0, NEG).astype(np.float32)
    mask01 = (k_idx <= q_idx).astype(np.float32)
    return np.concatenate([ident, maskb, mask01], axis=1)  # [P, 3P]


_PROG_CACHE = {}


# Matmul operand dtype: bf16 (default; 1 cycle/row on PE for ALL free-dim
# sizes, half the DMA bytes, ~2e-3 end-to-end rel err) or fp32r via
# BASS_MM_F32R=1 (~2e-4 err but 4 cycles/row whenever the matmul free dim
# < 256 — the PV and diagonal-score matmuls) or strict f32 via BASS_MM_F32=1.
USE_BF16 = (
    os.environ.get("BASS_MM_F32", "0") != "1"
    and os.environ.get("BASS_MM_F32R", "0") != "1"
)
USE_F32R = not USE_BF16 and os.environ.get("BASS_MM_F32", "0") != "1"


def _get_program(with_bias: bool):
    if with_bias not in _PROG_CACHE:
        _PROG_CACHE[with_bias] = build_program(
            with_bias, r_proj=USE_F32R, r_scores=USE_F32R, bf16=USE_BF16
        )
    return _PROG_CACHE[with_bias]


def make_in_maps(x, Wq, bq, Wk, bk, Wv, bv, Wo, bo, with_bias):
    if USE_BF16:
        import ml_dtypes

        in_dt = ml_dtypes.bfloat16
    else:
        in_dt = np.float32
    consts = _host_consts()
    maps = []
    for c in range(N_CORES):
        xc = np.ascontiguousarray(
            x[c * B_CORE : (c + 1) * B_CORE]  # [B_CORE, S, E]
            .reshape(TOK, E)
            .T  # [E, TOK]
        ).astype(in_dt)
        m = {
            "xt": xc,
            "wq": np.ascontiguousarray(np.asarray(Wq, dtype=np.float32).astype(in_dt)),
            "wk": np.ascontiguousarray(np.asarray(Wk, dtype=np.float32).astype(in_dt)),
            "wv": np.ascontiguousarray(np.asarray(Wv, dtype=np.float32).astype(in_dt)),
            "wo": np.ascontiguousarray(np.asarray(Wo, dtype=np.float32).astype(in_dt)),
            "consts": consts,
        }
        if with_bias:
            bqk = np.concatenate(
                [np.asarray(bq).reshape(KT, P).T, np.asarray(bk).reshape(KT, P).T],
                axis=1,
            ).astype(np.float32)
            bvb = np.zeros((P, H, D + 1), np.float32)
            bvb[:, :, :D] = np.broadcast_to(np.asarray(bv).reshape(H, D), (P, H, D))
            m["bqk"] = np.ascontiguousarray(bqk)
            m["bvb"] = np.ascontiguousarray(bvb.reshape(P, H * (D + 1)))
            m["bob"] = np.ascontiguousarray(
                np.broadcast_to(np.asarray(bo, dtype=np.float32), (P, E))
            )
        maps.append(m)
    return maps


def kernel(x, Wq, bq, Wk, bk, Wv, bv, Wo, bo):
    from concourse.bass_utils import run_bass_kernel_spmd

    x = np.asarray(x, dtype=np.float32)
    with_bias = any(
        float(np.abs(np.asarray(b)).max()) != 0.0 for b in (bq, bk, bv, bo)
    )
    nc = _get_program(with_bias)
    in_maps = make_in_maps(x, Wq, bq, Wk, bk, Wv, bv, Wo, bo, with_bias)
    res = run_bass_kernel_spmd(nc, in_maps, core_ids=list(range(N_CORES)))
    out = np.empty((B_FULL, S, E), dtype=np.float32)
    for c in range(N_CORES):
        out[c * B_CORE : (c + 1) * B_CORE] = res.results[c]["y"].reshape(B_CORE, S, E)
    return out

